# revision 3
# baseline (speedup 1.0000x reference)
"""Trainium2 Bass kernel for DynamicPTTopicModeling.

Computes, per batch b (one batch per NeuronCore, 8 cores):
    qg  = relu(qz @ bw.T)            # [R=8192, G=512], contraction over d=1024
    den = max(sum_g qg, 1e-6)        # per-row L1 norm
    msg = (qg @ bw) / den            # [R, D=1024]

Sharding: batch b across the 8 NeuronCores, fully data-parallel (one batch
per core, no collectives). Host pre-transposes qz/bw and converts to bf16
(tolerance 2e-2; measured pipeline error ~3e-3); the output is stored bf16
and upcast on the host. bf16 halves HBM traffic and, critically, halves
LDWEIGHTS time so the PE runs at the ~N-cycle matmul streaming floor
(f32 weights load at 2 cycles/col and capped the stream at 272 ns/MM).

Per-core schedule (16 mega-tiles of 512 rows, qzT loaded in 2-mega pairs so
DMA descriptor runs stay at 2 KB):
  - mm1 produces qg TRANSPOSED ([g, p] layout): stationary = bwT slices,
    moving = qzT chunks (N=512). mm2 consumes qg slices as its stationary
    with bw natural as moving.
  - The PE stream is software-pipelined one mega ahead: per iteration it
    runs mm1(t+1), then rowsum(t)/mm2(t). The tile scheduler pulls each
    matmul's LDWEIGHTS ~2 slots ahead of the MM, so without the interleave
    mm2's first group stalls on the ACT relu drain of mm1's last chunk
    (measured ~0.3-0.5 us per mega); with it, every mm2/rowsum dependency
    is ~36 MM-slots old and the PE never waits on ACT/DVE.
  - Row-sums over g (partition dim) via a ones-stationary matmul into
    [1, 512]; 4 tiny PE transposes flip it into [128, 4] column layout where
    max+reciprocal run lane-parallel; the scale is applied to the mm2 psum
    during the drain copy as a per-partition scalar multiply, alternating
    DVE/ACT so neither engine chokes.
"""
from contextlib import ExitStack

import numpy as np

import concourse.bass as bass
import concourse.tile as tile
from concourse import bacc, mybir
from concourse.bass_utils import run_bass_kernel_spmd

F32 = mybir.dt.float32
BF16 = mybir.dt.bfloat16
AF = mybir.ActivationFunctionType

B, C, P, D, G = 8, 16, 512, 1024, 512
R = C * P            # 8192 rows per batch
MEGA = 512           # rows per mega-tile
NSUB = MEGA // 128   # 4
NMEGA = R // MEGA    # 16
NPAIR = NMEGA // 2   # 8 (qzT loads are 2 megas per tile: 2KB runs per (p,k))
KD = D // 128        # 8 d-chunks
KG = G // 128        # 4 g-chunks
EPS = 1e-6
N_CORES = 8


def build_kernel():
    nc = bacc.Bacc("TRN2", target_bir_lowering=False)
    qzT_d = nc.dram_tensor("qzT", [D, R], BF16, kind="ExternalInput")
    bw_d = nc.dram_tensor("bw", [G, D], BF16, kind="ExternalInput")
    bwT_d = nc.dram_tensor("bwT", [D, G], BF16, kind="ExternalInput")
    msg_d = nc.dram_tensor("msg", [R, D], BF16, kind="ExternalOutput")

    with tile.TileContext(nc) as tc, ExitStack() as ctx:
        const_pool = ctx.enter_context(tc.tile_pool(name="const", bufs=1))
        in_pool = ctx.enter_context(tc.tile_pool(name="inp", bufs=3))
        qgr_pool = ctx.enter_context(tc.tile_pool(name="qgrp", bufs=2))
        out_pool = ctx.enter_context(tc.tile_pool(name="outp", bufs=2))
        small_pool = ctx.enter_context(tc.tile_pool(name="smallp", bufs=2))
        qg_psum = ctx.enter_context(tc.tile_pool(name="qgps", bufs=2, space="PSUM"))
        msg_psum = ctx.enter_context(tc.tile_pool(name="msgps", bufs=6, space="PSUM"))

        ones_f = const_pool.tile([128, 1], F32)
        nc.vector.memset(ones_f, 1.0)
        ones_g = const_pool.tile([128, 1], BF16)
        nc.vector.tensor_copy(ones_g, ones_f)
        one_e = const_pool.tile([1, 1], F32)
        nc.vector.memset(one_e, 1.0)

        # Weights on the second HWDGE ring (nc.scalar). bwT in single
        # k-slices (128KB) so the very first matmul only waits ~0.4us.
        bwT_sb = const_pool.tile([128, KD, G], BF16)
        bwT_view = bwT_d[:].rearrange("(k p) g -> p k g", p=128)
        for k in range(KD):
            nc.scalar.dma_start(
                out=bwT_sb[:, k:k + 1, :], in_=bwT_view[:, k:k + 1, :]
            )
        bw_sb = const_pool.tile([128, KG, D], BF16)
        nc.scalar.dma_start(
            out=bw_sb, in_=bw_d[:].rearrange("(gc p) d -> p gc d", p=128)
        )

        def load_qzT(j):
            # one tile = 2 megas (1024 rows): per-(p,k) DRAM runs are 2KB.
            # pair 0 loads in single k-slices so the first matmul starts at
            # the earliest possible moment; later pairs are one DMA each.
            qzT = in_pool.tile([128, KD, 2 * MEGA], BF16, name="qzT")
            qzT_view = qzT_d[:, j * 2 * MEGA:(j + 1) * 2 * MEGA].rearrange(
                "(k p) r -> p k r", p=128
            )
            nq = 8 if j == 0 else 1
            step = KD // nq
            for q in range(nq):
                nc.sync.dma_start(
                    out=qzT[:, step * q:step * (q + 1), :],
                    in_=qzT_view[:, step * q:step * (q + 1), :],
                )
            return qzT

        pairs = {}

        def ensure_load(j):
            if 0 <= j < NPAIR and j not in pairs:
                pairs[j] = load_qzT(j)

        def mm1(t):
            # qgT[gc] = sum_k bwT[:,k,gc].T @ qzT[:,k,cols(t)] -> relu (ACT)
            qzT = pairs[t // 2]
            c0 = (t % 2) * MEGA
            qgr = qgr_pool.tile([128, KG, MEGA], BF16, name="qgr")
            for gc in range(KG):
                qg_ps = qg_psum.tile([128, MEGA], F32, name="qg_ps")
                for k in range(KD):
                    nc.tensor.matmul(
                        qg_ps,
                        bwT_sb[:, k, gc * 128:(gc + 1) * 128],
                        qzT[:, k, c0:c0 + MEGA],
                        start=(k == 0),
                        stop=(k == KD - 1),
                    )
                nc.scalar.activation(qgr[:, gc, :], qg_ps, AF.Relu)
            return qgr

        def mm2_block(t, qgr):
            msg_sb = out_pool.tile([128, NSUB, D], BF16, name="msg_sb")

            def mmgroup(s, h):
                m_ps = msg_psum.tile([128, 512], F32, name="m_ps")
                for gc in range(KG):
                    nc.tensor.matmul(
                        m_ps,
                        qgr[:, gc, s * 128:(s + 1) * 128],
                        bw_sb[:, gc, h * 512:(h + 1) * 512],
                        start=(gc == 0),
                        stop=(gc == KG - 1),
                    )
                return m_ps

            def drain(s, h, m_ps, sc_sb):
                # alternate DVE/ACT so neither engine is the choke point
                dst = msg_sb[:, s, h * 512:(h + 1) * 512]
                if (s * 2 + h) % 2 == 0:
                    nc.vector.tensor_scalar_mul(dst, m_ps, sc_sb[:, s:s + 1])
                else:
                    nc.scalar.mul(dst, m_ps, sc_sb[:, s:s + 1])

            pending = [(0, 0, mmgroup(0, 0))]

            # rowsum over g via ones-stationary MM (rs/sc psums share the
            # qg pool's slots; their previous occupants were drained by the
            # mm1(t+1) relus that ran ~4 MM-groups ago)
            rs_ps = qg_psum.tile([1, MEGA], F32, name="rs_ps", tag="qg_ps")
            for gc in range(KG):
                nc.tensor.matmul(
                    rs_ps,
                    ones_g,
                    qgr[:, gc, :],
                    start=(gc == 0),
                    stop=(gc == KG - 1),
                )
            rs_sb = small_pool.tile([1, MEGA], F32, name="rs_sb")
            nc.vector.tensor_copy(rs_sb, rs_ps)

            pending.append((0, 1, mmgroup(0, 1)))

            # rowsum -> column layout via tiny PE transposes (the DVE copy
            # of rs_sb runs under group (0,1))
            sc_ps = qg_psum.tile([128, NSUB], F32, name="sc_ps", tag="qg_ps")
            for ss in range(NSUB):
                nc.tensor.matmul(
                    sc_ps[:, ss:ss + 1],
                    rs_sb[0:1, ss * 128:(ss + 1) * 128],
                    one_e,
                    is_transpose=True,
                )

            pending.append((1, 0, mmgroup(1, 0)))

            sc_sb = small_pool.tile([128, NSUB], F32, name="sc_sb")
            nc.vector.tensor_scalar_max(sc_sb, sc_ps, EPS)
            nc.vector.reciprocal(sc_sb, sc_sb)

            pending.append((1, 1, mmgroup(1, 1)))
            for (ps_, hs_, mp_) in pending:
                drain(ps_, hs_, mp_, sc_sb)

            last = t == NMEGA - 1
            if last:
                for s in (0, 1):
                    nc.sync.dma_start(
                        out=msg_d[t * MEGA + s * 128:t * MEGA + (s + 1) * 128, :],
                        in_=msg_sb[:, s, :],
                    )
            else:
                nc.sync.dma_start(
                    out=msg_d[t * MEGA:t * MEGA + 256, :].rearrange(
                        "(s p) d -> p s d", p=128
                    ),
                    in_=msg_sb[:, 0:2, :],
                )
            for s in (2, 3):
                for h in (0, 1):
                    drain(s, h, mmgroup(s, h), sc_sb)
                if last:
                    # per-sub stores at the end: the final store is only
                    # 256KB, shrinking the post-compute tail
                    nc.sync.dma_start(
                        out=msg_d[t * MEGA + s * 128:t * MEGA + (s + 1) * 128, :],
                        in_=msg_sb[:, s, :],
                    )
            if not last:
                nc.sync.dma_start(
                    out=msg_d[t * MEGA + 256:(t + 1) * MEGA, :].rearrange(
                        "(s p) d -> p s d", p=128
                    ),
                    in_=msg_sb[:, 2:4, :],
                )

        ensure_load(0)
        ensure_load(1)
        ensure_load(2)
        qgr_cur = mm1(0)
        for t in range(NMEGA):
            if t + 1 < NMEGA:
                if (t + 1) % 2 == 0:
                    ensure_load((t + 1) // 2 + 2)
                qgr_next = mm1(t + 1)
            else:
                qgr_next = None
            mm2_block(t, qgr_cur)
            qgr_cur = qgr_next

    nc.compile()
    return nc


_NC_CACHE = None


def _get_nc():
    global _NC_CACHE
    if _NC_CACHE is None:
        _NC_CACHE = build_kernel()
    return _NC_CACHE


def kernel(qz: np.ndarray, binary_weight: np.ndarray) -> np.ndarray:
    import ml_dtypes

    bf16 = ml_dtypes.bfloat16
    qz = np.asarray(qz, dtype=np.float32)
    bw32 = np.asarray(binary_weight, dtype=np.float32)
    assert qz.shape == (B, C, P, D), qz.shape
    assert bw32.shape == (B, G, D), bw32.shape
    bw = bw32.astype(bf16)

    nc = _get_nc()
    in_maps = []
    for i in range(N_CORES):
        qzT = np.ascontiguousarray(qz[i].reshape(R, D).T.astype(bf16))  # [D, R]
        bwT = np.ascontiguousarray(bw[i].T)                             # [D, G]
        in_maps.append({"qzT": qzT, "bw": bw[i], "bwT": bwT})
    res = run_bass_kernel_spmd(nc, in_maps, core_ids=list(range(N_CORES)))
    out = np.stack(
        [
            res.results[i]["msg"].astype(np.float32).reshape(C, P, D)
            for i in range(N_CORES)
        ],
        axis=0,
    )
    return out


# revision 7
# speedup vs baseline: 1.0137x; 1.0137x over previous
"""Trainium2 Bass kernel for DynamicPTTopicModeling.

Computes, per batch b (one batch per NeuronCore, 8 cores):
    qg  = relu(qz @ bw.T)            # [R=8192, G=512], contraction over d=1024
    den = max(sum_g qg, 1e-6)        # per-row L1 norm
    msg = (qg @ bw) / den            # [R, D=1024]

Sharding: batch b across the 8 NeuronCores, fully data-parallel (one batch
per core, no collectives). Host pre-transposes qz/bw and converts to bf16
(tolerance 2e-2; measured pipeline error ~3e-3); the output is stored bf16
and upcast on the host. bf16 halves HBM traffic and, critically, halves
LDWEIGHTS time so the PE runs at the ~N-cycle matmul streaming floor
(f32 weights load at 2 cycles/col and capped the stream at 272 ns/MM).

Per-core schedule (16 mega-tiles of 512 rows, qzT loaded in 2-mega pairs so
DMA descriptor runs stay at 2 KB):
  - mm1 produces qg TRANSPOSED ([g, p] layout): stationary = bwT slices,
    moving = qzT chunks (N=512). mm2 consumes qg slices as its stationary
    with bw natural as moving.
  - The PE stream is software-pipelined one mega ahead: per iteration it
    runs mm1(t+1), then rowsum(t)/mm2(t). The tile scheduler pulls each
    matmul's LDWEIGHTS ~2 slots ahead of the MM, so without the interleave
    mm2's first group stalls on the ACT relu drain of mm1's last chunk
    (measured ~0.3-0.5 us per mega); with it, every mm2/rowsum dependency
    is ~36 MM-slots old and the PE never waits on ACT/DVE.
  - Row-sums over g (partition dim) via a ones-stationary matmul into
    [1, 512]; 4 tiny PE transposes flip it into [128, 4] column layout where
    max+reciprocal run lane-parallel; the scale is applied to the mm2 psum
    during the drain copy as a per-partition scalar multiply, alternating
    DVE/ACT so neither engine chokes.
"""
from contextlib import ExitStack

import numpy as np

import concourse.bass as bass
import concourse.tile as tile
from concourse import bacc, mybir
from concourse.bass_utils import run_bass_kernel_spmd

F32 = mybir.dt.float32
BF16 = mybir.dt.bfloat16
AF = mybir.ActivationFunctionType

B, C, P, D, G = 8, 16, 512, 1024, 512
R = C * P            # 8192 rows per batch
MEGA = 512           # rows per mega-tile
NSUB = MEGA // 128   # 4
NMEGA = R // MEGA    # 16
NPAIR = NMEGA // 2   # 8 (qzT loads are 2 megas per tile: 2KB runs per (p,k))
KD = D // 128        # 8 d-chunks
KG = G // 128        # 4 g-chunks
EPS = 1e-6
N_CORES = 8


def build_kernel():
    nc = bacc.Bacc("TRN2", target_bir_lowering=False)
    qzT_d = nc.dram_tensor("qzT", [D, R], BF16, kind="ExternalInput")
    bw_d = nc.dram_tensor("bw", [G, D], BF16, kind="ExternalInput")
    bwT_d = nc.dram_tensor("bwT", [D, G], BF16, kind="ExternalInput")
    msg_d = nc.dram_tensor("msg", [R, D], BF16, kind="ExternalOutput")

    with tile.TileContext(nc) as tc, ExitStack() as ctx:
        const_pool = ctx.enter_context(tc.tile_pool(name="const", bufs=1))
        in_pool = ctx.enter_context(tc.tile_pool(name="inp", bufs=3))
        qgr_pool = ctx.enter_context(tc.tile_pool(name="qgrp", bufs=2))
        out_pool = ctx.enter_context(tc.tile_pool(name="outp", bufs=2))
        small_pool = ctx.enter_context(tc.tile_pool(name="smallp", bufs=2))
        qg_psum = ctx.enter_context(tc.tile_pool(name="qgps", bufs=2, space="PSUM"))
        msg_psum = ctx.enter_context(tc.tile_pool(name="msgps", bufs=6, space="PSUM"))

        ones_f = const_pool.tile([128, 1], F32)
        nc.vector.memset(ones_f, 1.0)
        ones_g = const_pool.tile([128, 1], BF16)
        nc.vector.tensor_copy(ones_g, ones_f)
        one_e = const_pool.tile([1, 1], F32)
        nc.vector.memset(one_e, 1.0)

        # Weights on the second HWDGE ring (nc.scalar). bwT in k-quarters:
        # finer slicing costs ~0.7us of ring issue time per extra DMA, which
        # outweighs the granularity win.
        bwT_sb = const_pool.tile([128, KD, G], BF16)
        bwT_view = bwT_d[:].rearrange("(k p) g -> p k g", p=128)
        for q in range(4):
            nc.scalar.dma_start(
                out=bwT_sb[:, 2 * q:2 * q + 2, :], in_=bwT_view[:, 2 * q:2 * q + 2, :]
            )
        bw_sb = const_pool.tile([128, KG, D], BF16)
        nc.scalar.dma_start(
            out=bw_sb, in_=bw_d[:].rearrange("(gc p) d -> p gc d", p=128)
        )

        # ~10 junk matmuls on memset tiles bridge the DMA-bound load window
        # so the PE_HAM clock gate sees >=3.4us of sustained activity and the
        # real matmul stream starts at 2.4 GHz instead of warming up mid-way.
        warm_a = const_pool.tile([128, 128], BF16)
        nc.vector.memset(warm_a, 0.0)
        warm_b = const_pool.tile([128, 512], BF16)
        nc.vector.memset(warm_b, 0.0)
        warm_ps = msg_psum.tile([128, 512], F32, name="warm_ps", tag="m_ps")
        for _ in range(10):
            nc.tensor.matmul(warm_ps, warm_a, warm_b)

        def load_qzT(j):
            # one tile = 2 megas (1024 rows): per-(p,k) DRAM runs are 2KB.
            # pair 0 loads in single k-slices so the first matmul starts at
            # the earliest possible moment; later pairs are one DMA each.
            qzT = in_pool.tile([128, KD, 2 * MEGA], BF16, name="qzT")
            qzT_view = qzT_d[:, j * 2 * MEGA:(j + 1) * 2 * MEGA].rearrange(
                "(k p) r -> p k r", p=128
            )
            nq = 4 if j == 0 else 1
            step = KD // nq
            for q in range(nq):
                nc.sync.dma_start(
                    out=qzT[:, step * q:step * (q + 1), :],
                    in_=qzT_view[:, step * q:step * (q + 1), :],
                )
            return qzT

        pairs = {}

        def ensure_load(j):
            if 0 <= j < NPAIR and j not in pairs:
                pairs[j] = load_qzT(j)

        def mm1(t):
            # qgT[gc] = sum_k bwT[:,k,gc].T @ qzT[:,k,cols(t)] -> relu (ACT)
            qzT = pairs[t // 2]
            c0 = (t % 2) * MEGA
            qgr = qgr_pool.tile([128, KG, MEGA], BF16, name="qgr")
            for gc in range(KG):
                qg_ps = qg_psum.tile([128, MEGA], F32, name="qg_ps")
                for k in range(KD):
                    nc.tensor.matmul(
                        qg_ps,
                        bwT_sb[:, k, gc * 128:(gc + 1) * 128],
                        qzT[:, k, c0:c0 + MEGA],
                        start=(k == 0),
                        stop=(k == KD - 1),
                    )
                nc.scalar.activation(qgr[:, gc, :], qg_ps, AF.Relu)
            return qgr

        def mm2_block(t, qgr):
            msg_sb = out_pool.tile([128, NSUB, D], BF16, name="msg_sb")

            def mmgroup(s, h):
                m_ps = msg_psum.tile([128, 512], F32, name="m_ps")
                for gc in range(KG):
                    nc.tensor.matmul(
                        m_ps,
                        qgr[:, gc, s * 128:(s + 1) * 128],
                        bw_sb[:, gc, h * 512:(h + 1) * 512],
                        start=(gc == 0),
                        stop=(gc == KG - 1),
                    )
                return m_ps

            def drain(s, h, m_ps, sc_sb):
                # all drains on DVE: ACT only runs the relus, so a drain is
                # never queued behind the next mega's relus on ACT's strict
                # FIFO (that ordering stalled mm2 psum-slot reuse by ~3us)
                dst = msg_sb[:, s, h * 512:(h + 1) * 512]
                nc.vector.tensor_scalar_mul(dst, m_ps, sc_sb[:, s:s + 1])

            pending = [(0, 0, mmgroup(0, 0))]

            # rowsum over g via ones-stationary MM (rs/sc psums share the
            # qg pool's slots; their previous occupants were drained by the
            # mm1(t+1) relus that ran ~4 MM-groups ago)
            rs_ps = qg_psum.tile([1, MEGA], F32, name="rs_ps", tag="qg_ps")
            for gc in range(KG):
                nc.tensor.matmul(
                    rs_ps,
                    ones_g,
                    qgr[:, gc, :],
                    start=(gc == 0),
                    stop=(gc == KG - 1),
                )
            rs_sb = small_pool.tile([1, MEGA], F32, name="rs_sb")
            nc.vector.tensor_copy(rs_sb, rs_ps)

            pending.append((0, 1, mmgroup(0, 1)))

            # rowsum -> column layout via tiny PE transposes (the DVE copy
            # of rs_sb runs under group (0,1))
            sc_ps = qg_psum.tile([128, NSUB], F32, name="sc_ps", tag="qg_ps")
            for ss in range(NSUB):
                nc.tensor.matmul(
                    sc_ps[:, ss:ss + 1],
                    rs_sb[0:1, ss * 128:(ss + 1) * 128],
                    one_e,
                    is_transpose=True,
                )

            pending.append((1, 0, mmgroup(1, 0)))

            sc_sb = small_pool.tile([128, NSUB], F32, name="sc_sb")
            nc.vector.tensor_scalar_max(sc_sb, sc_ps, EPS)
            nc.vector.reciprocal(sc_sb, sc_sb)

            pending.append((1, 1, mmgroup(1, 1)))
            for (ps_, hs_, mp_) in pending:
                drain(ps_, hs_, mp_, sc_sb)

            last = t == NMEGA - 1
            if last:
                for s in (0, 1):
                    nc.sync.dma_start(
                        out=msg_d[t * MEGA + s * 128:t * MEGA + (s + 1) * 128, :],
                        in_=msg_sb[:, s, :],
                    )
            else:
                nc.sync.dma_start(
                    out=msg_d[t * MEGA:t * MEGA + 256, :].rearrange(
                        "(s p) d -> p s d", p=128
                    ),
                    in_=msg_sb[:, 0:2, :],
                )
            for s in (2, 3):
                for h in (0, 1):
                    drain(s, h, mmgroup(s, h), sc_sb)
                if last:
                    # per-sub stores at the end: the final store is only
                    # 256KB, shrinking the post-compute tail
                    nc.sync.dma_start(
                        out=msg_d[t * MEGA + s * 128:t * MEGA + (s + 1) * 128, :],
                        in_=msg_sb[:, s, :],
                    )
            if not last:
                nc.sync.dma_start(
                    out=msg_d[t * MEGA + 256:(t + 1) * MEGA, :].rearrange(
                        "(s p) d -> p s d", p=128
                    ),
                    in_=msg_sb[:, 2:4, :],
                )

        ensure_load(0)
        ensure_load(1)
        ensure_load(2)
        qgr_cur = mm1(0)
        for t in range(NMEGA):
            if t + 1 < NMEGA:
                if (t + 1) % 2 == 0:
                    ensure_load((t + 1) // 2 + 2)
                qgr_next = mm1(t + 1)
            else:
                qgr_next = None
            mm2_block(t, qgr_cur)
            qgr_cur = qgr_next

    nc.compile()
    return nc


_NC_CACHE = None


def _get_nc():
    global _NC_CACHE
    if _NC_CACHE is None:
        _NC_CACHE = build_kernel()
    return _NC_CACHE


def kernel(qz: np.ndarray, binary_weight: np.ndarray) -> np.ndarray:
    import ml_dtypes

    bf16 = ml_dtypes.bfloat16
    qz = np.asarray(qz, dtype=np.float32)
    bw32 = np.asarray(binary_weight, dtype=np.float32)
    assert qz.shape == (B, C, P, D), qz.shape
    assert bw32.shape == (B, G, D), bw32.shape
    bw = bw32.astype(bf16)

    nc = _get_nc()
    in_maps = []
    for i in range(N_CORES):
        qzT = np.ascontiguousarray(qz[i].reshape(R, D).T.astype(bf16))  # [D, R]
        bwT = np.ascontiguousarray(bw[i].T)                             # [D, G]
        in_maps.append({"qzT": qzT, "bw": bw[i], "bwT": bwT})
    res = run_bass_kernel_spmd(nc, in_maps, core_ids=list(range(N_CORES)))
    out = np.stack(
        [
            res.results[i]["msg"].astype(np.float32).reshape(C, P, D)
            for i in range(N_CORES)
        ],
        axis=0,
    )
    return out


# revision 12
# speedup vs baseline: 1.0908x; 1.0760x over previous
"""Trainium2 Bass kernel for DynamicPTTopicModeling.

Computes, per batch b (one batch per NeuronCore, 8 cores):
    qg  = relu(qz @ bw.T)            # [R=8192, G=512], contraction over d=1024
    den = max(sum_g qg, 1e-6)        # per-row L1 norm
    msg = (qg @ bw) / den            # [R, D=1024]

Sharding: batch b across the 8 NeuronCores, fully data-parallel (one batch
per core, no collectives). Host pre-transposes qz/bw and converts to bf16
(tolerance 2e-2; measured pipeline error ~3e-3); the output is stored bf16
and upcast on the host. bf16 halves HBM traffic and, critically, halves
LDWEIGHTS time so the PE runs at the ~N-cycle matmul streaming floor
(f32 weights load at 2 cycles/col and capped the stream at 272 ns/MM).

Per-core schedule (16 mega-tiles of 512 rows, qzT loaded in 2-mega pairs so
DMA descriptor runs stay at 2 KB):
  - mm1 produces qg TRANSPOSED ([g, p] layout): stationary = bwT slices,
    moving = qzT chunks (N=512). mm2 consumes qg slices as its stationary
    with bw natural as moving.
  - The PE stream is software-pipelined one mega ahead: per iteration it
    runs mm1(t+1), then rowsum(t)/mm2(t). The tile scheduler pulls each
    matmul's LDWEIGHTS ~2 slots ahead of the MM, so without the interleave
    mm2's first group stalls on the ACT relu drain of mm1's last chunk
    (measured ~0.3-0.5 us per mega); with it, every mm2/rowsum dependency
    is ~36 MM-slots old and the PE never waits on ACT/DVE.
  - Row-sums over g (partition dim) via a ones-stationary matmul into
    [1, 512]; 4 tiny PE transposes flip it into [128, 4] column layout where
    max+reciprocal run lane-parallel; the scale is applied to the mm2 psum
    during the drain copy as a per-partition scalar multiply, alternating
    DVE/ACT so neither engine chokes.
"""
from contextlib import ExitStack

import numpy as np

import concourse.bass as bass
import concourse.tile as tile
from concourse import bacc, mybir
from concourse.bass_utils import run_bass_kernel_spmd

F32 = mybir.dt.float32
BF16 = mybir.dt.bfloat16
AF = mybir.ActivationFunctionType

B, C, P, D, G = 8, 16, 512, 1024, 512
R = C * P            # 8192 rows per batch
MEGA = 512           # rows per mega-tile
NSUB = MEGA // 128   # 4
NMEGA = R // MEGA    # 16
NPAIR = NMEGA // 2   # 8 (qzT loads are 2 megas per tile: 2KB runs per (p,k))
KD = D // 128        # 8 d-chunks
KG = G // 128        # 4 g-chunks
EPS = 1e-6
N_CORES = 8


def build_kernel():
    nc = bacc.Bacc("TRN2", target_bir_lowering=False)
    qzT_d = nc.dram_tensor("qzT", [D, R], BF16, kind="ExternalInput")
    bw_d = nc.dram_tensor("bw", [G, D], BF16, kind="ExternalInput")
    bwT_d = nc.dram_tensor("bwT", [D, G], BF16, kind="ExternalInput")
    msg_d = nc.dram_tensor("msg", [R, D], BF16, kind="ExternalOutput")

    with tile.TileContext(nc) as tc, ExitStack() as ctx:
        const_pool = ctx.enter_context(tc.tile_pool(name="const", bufs=1))
        in_pool = ctx.enter_context(tc.tile_pool(name="inp", bufs=3))
        qgr_pool = ctx.enter_context(tc.tile_pool(name="qgrp", bufs=2))
        out_pool = ctx.enter_context(tc.tile_pool(name="outp", bufs=2))
        small_pool = ctx.enter_context(tc.tile_pool(name="smallp", bufs=2))
        qg_psum = ctx.enter_context(tc.tile_pool(name="qgps", bufs=2, space="PSUM"))
        msg_psum = ctx.enter_context(tc.tile_pool(name="msgps", bufs=6, space="PSUM"))

        ones_f = const_pool.tile([128, 1], F32)
        nc.vector.memset(ones_f, 1.0)
        ones_g = const_pool.tile([128, 1], BF16)
        nc.vector.tensor_copy(ones_g, ones_f)

        # Weights on the second HWDGE ring (nc.scalar). bwT in k-quarters:
        # finer slicing costs ~0.7us of ring issue time per extra DMA, which
        # outweighs the granularity win.
        bwT_sb = const_pool.tile([128, KD, G], BF16)
        bwT_view = bwT_d[:].rearrange("(k p) g -> p k g", p=128)
        for q in range(4):
            nc.scalar.dma_start(
                out=bwT_sb[:, 2 * q:2 * q + 2, :], in_=bwT_view[:, 2 * q:2 * q + 2, :]
            )
        bw_sb = const_pool.tile([128, KG, D], BF16)
        nc.scalar.dma_start(
            out=bw_sb, in_=bw_d[:].rearrange("(gc p) d -> p gc d", p=128)
        )

        # ~10 junk matmuls on memset tiles bridge the DMA-bound load window
        # so the PE_HAM clock gate sees >=3.4us of sustained activity and the
        # real matmul stream starts at 2.4 GHz instead of warming up mid-way.
        warm_a = const_pool.tile([128, 128], BF16)
        nc.vector.memset(warm_a, 0.0)
        warm_b = const_pool.tile([128, 512], BF16)
        nc.vector.memset(warm_b, 0.0)
        # 22 MMs ~= 4.7us: enough to cover until the first qzT quarters can
        # physically arrive (~12.4us: 7.2us engine-start barrier + 2MB over
        # one HWDGE ring), without delaying the first real matmul past that.
        warm_ps = msg_psum.tile([128, 512], F32, name="warm_ps", tag="m_ps")
        for _ in range(22):
            nc.tensor.matmul(warm_ps, warm_a, warm_b)

        def load_qzT(j):
            # one tile = 2 megas (1024 rows): per-(p,k) DRAM runs are 2KB.
            # pair 0 loads in single k-slices so the first matmul starts at
            # the earliest possible moment; later pairs are one DMA each.
            qzT = in_pool.tile([128, KD, 2 * MEGA], BF16, name="qzT")
            qzT_view = qzT_d[:, j * 2 * MEGA:(j + 1) * 2 * MEGA].rearrange(
                "(k p) r -> p k r", p=128
            )
            nq = 4 if j == 0 else 1
            step = KD // nq
            for q in range(nq):
                nc.sync.dma_start(
                    out=qzT[:, step * q:step * (q + 1), :],
                    in_=qzT_view[:, step * q:step * (q + 1), :],
                )
            return qzT

        pairs = {}

        def ensure_load(j):
            if 0 <= j < NPAIR and j not in pairs:
                pairs[j] = load_qzT(j)

        def mm1(t):
            # qgT[gc] = sum_k bwT[:,k,gc].T @ qzT[:,k,cols(t)] -> relu (ACT)
            qzT = pairs[t // 2]
            c0 = (t % 2) * MEGA
            qgr = qgr_pool.tile([128, KG, MEGA], BF16, name="qgr")
            for gc in range(KG):
                qg_ps = qg_psum.tile([128, MEGA], F32, name="qg_ps")
                for k in range(KD):
                    nc.tensor.matmul(
                        qg_ps,
                        bwT_sb[:, k, gc * 128:(gc + 1) * 128],
                        qzT[:, k, c0:c0 + MEGA],
                        start=(k == 0),
                        stop=(k == KD - 1),
                    )
                nc.scalar.activation(qgr[:, gc, :], qg_ps, AF.Relu)
            return qgr

        def mm2_block(t, qgr):
            msg_sb = out_pool.tile([128, NSUB, D], BF16, name="msg_sb")

            def mmgroup(s, h):
                m_ps = msg_psum.tile([128, 512], F32, name="m_ps")
                for gc in range(KG):
                    nc.tensor.matmul(
                        m_ps,
                        qgr[:, gc, s * 128:(s + 1) * 128],
                        bw_sb[:, gc, h * 512:(h + 1) * 512],
                        start=(gc == 0),
                        stop=(gc == KG - 1),
                    )
                return m_ps

            def drain(s, h, m_ps, sc_sb):
                # all drains on DVE: ACT only runs the relus, so a drain is
                # never queued behind the next mega's relus on ACT's strict
                # FIFO (that ordering stalled mm2 psum-slot reuse by ~3us)
                dst = msg_sb[:, s, h * 512:(h + 1) * 512]
                nc.vector.tensor_scalar_mul(dst, m_ps, sc_sb[:, s:s + 1])

            # rowsum over g, den-direct: DVE sums the 4 qgr chunks into
            # acc [128(g_low), p] (bf16, error ~1e-3 of den — negligible),
            # then 4 tiny N=1 matmuls acc_chunk.T @ ones produce den for
            # each 128-row sub ALREADY in per-partition column layout.
            # This replaces 4 N=512 rowsum MMs + 4 PE transposes + a DVE
            # copy (~1.2us of PE per mega) with ~0.35us of PE.
            ADD = mybir.AluOpType.add
            s1 = small_pool.tile([128, MEGA], BF16, name="acc_s1")
            nc.vector.scalar_tensor_tensor(s1, qgr[:, 0, :], 0.0, qgr[:, 1, :], ADD, ADD)
            s2 = small_pool.tile([128, MEGA], BF16, name="acc_s2")
            nc.vector.scalar_tensor_tensor(s2, qgr[:, 2, :], 0.0, qgr[:, 3, :], ADD, ADD)
            acc = small_pool.tile([128, MEGA], BF16, name="acc")
            nc.vector.scalar_tensor_tensor(acc, s1, 0.0, s2, ADD, ADD)

            pending = [(0, 0, mmgroup(0, 0))]

            sc_ps = qg_psum.tile([128, NSUB], F32, name="sc_ps", tag="qg_ps")
            for ss in range(NSUB):
                nc.tensor.matmul(
                    sc_ps[:, ss:ss + 1],
                    acc[:, ss * 128:(ss + 1) * 128],
                    ones_g,
                )

            pending.append((0, 1, mmgroup(0, 1)))

            sc_sb = small_pool.tile([128, NSUB], F32, name="sc_sb")
            nc.vector.tensor_scalar_max(sc_sb, sc_ps, EPS)
            nc.vector.reciprocal(sc_sb, sc_sb)

            pending.append((1, 0, mmgroup(1, 0)))

            pending.append((1, 1, mmgroup(1, 1)))
            for (ps_, hs_, mp_) in pending:
                drain(ps_, hs_, mp_, sc_sb)

            last = t == NMEGA - 1
            if last:
                for s in (0, 1):
                    nc.sync.dma_start(
                        out=msg_d[t * MEGA + s * 128:t * MEGA + (s + 1) * 128, :],
                        in_=msg_sb[:, s, :],
                    )
            for s in (2, 3):
                for h in (0, 1):
                    drain(s, h, mmgroup(s, h), sc_sb)
                if last:
                    # per-sub stores at the end: the final store is only
                    # 256KB, shrinking the post-compute tail
                    nc.sync.dma_start(
                        out=msg_d[t * MEGA + s * 128:t * MEGA + (s + 1) * 128, :],
                        in_=msg_sb[:, s, :],
                    )
            if not last:
                # one store per mega: fewer ring-issue slots and completion
                # semaphores (the teardown epilogue scales with DMA count)
                nc.sync.dma_start(
                    out=msg_d[t * MEGA:(t + 1) * MEGA, :].rearrange(
                        "(s p) d -> p s d", p=128
                    ),
                    in_=msg_sb,
                )

        ensure_load(0)
        ensure_load(1)
        ensure_load(2)
        qgr_cur = mm1(0)
        for t in range(NMEGA):
            if t + 1 < NMEGA:
                if (t + 1) % 2 == 0:
                    ensure_load((t + 1) // 2 + 2)
                qgr_next = mm1(t + 1)
            else:
                qgr_next = None
            mm2_block(t, qgr_cur)
            qgr_cur = qgr_next

    nc.compile()
    return nc


_NC_CACHE = None


def _get_nc():
    global _NC_CACHE
    if _NC_CACHE is None:
        _NC_CACHE = build_kernel()
    return _NC_CACHE


def kernel(qz: np.ndarray, binary_weight: np.ndarray) -> np.ndarray:
    import ml_dtypes

    bf16 = ml_dtypes.bfloat16
    qz = np.asarray(qz, dtype=np.float32)
    bw32 = np.asarray(binary_weight, dtype=np.float32)
    assert qz.shape == (B, C, P, D), qz.shape
    assert bw32.shape == (B, G, D), bw32.shape
    bw = bw32.astype(bf16)

    nc = _get_nc()
    in_maps = []
    for i in range(N_CORES):
        qzT = np.ascontiguousarray(qz[i].reshape(R, D).T.astype(bf16))  # [D, R]
        bwT = np.ascontiguousarray(bw[i].T)                             # [D, G]
        in_maps.append({"qzT": qzT, "bw": bw[i], "bwT": bwT})
    res = run_bass_kernel_spmd(nc, in_maps, core_ids=list(range(N_CORES)))
    out = np.stack(
        [
            res.results[i]["msg"].astype(np.float32).reshape(C, P, D)
            for i in range(N_CORES)
        ],
        axis=0,
    )
    return out


# revision 17
# speedup vs baseline: 1.0971x; 1.0058x over previous
"""Trainium2 Bass kernel for DynamicPTTopicModeling.

Computes, per batch b (one batch per NeuronCore, 8 cores):
    qg  = relu(qz @ bw.T)            # [R=8192, G=512], contraction over d=1024
    den = max(sum_g qg, 1e-6)        # per-row L1 norm
    msg = (qg @ bw) / den            # [R, D=1024]

Sharding: batch b across the 8 NeuronCores, fully data-parallel (one batch
per core, no collectives). Host pre-transposes qz/bw and converts to bf16
(tolerance 2e-2; measured pipeline error ~3e-3); the output is stored bf16
and upcast on the host. bf16 halves HBM traffic and, critically, halves
LDWEIGHTS time so the PE runs at the ~N-cycle matmul streaming floor
(f32 weights load at 2 cycles/col and capped the stream at 272 ns/MM).

Per-core schedule (16 mega-tiles of 512 rows, qzT loaded in 2-mega pairs so
DMA descriptor runs stay at 2 KB):
  - mm1 produces qg TRANSPOSED ([g, p] layout): stationary = bwT slices,
    moving = qzT chunks (N=512). mm2 consumes qg slices as its stationary
    with bw natural as moving.
  - The PE stream is software-pipelined one mega ahead: per iteration it
    runs mm1(t+1), then rowsum(t)/mm2(t). The tile scheduler pulls each
    matmul's LDWEIGHTS ~2 slots ahead of the MM, so without the interleave
    mm2's first group stalls on the ACT relu drain of mm1's last chunk
    (measured ~0.3-0.5 us per mega); with it, every mm2/rowsum dependency
    is ~36 MM-slots old and the PE never waits on ACT/DVE.
  - Row-sums over g (partition dim) via a ones-stationary matmul into
    [1, 512]; 4 tiny PE transposes flip it into [128, 4] column layout where
    max+reciprocal run lane-parallel; the scale is applied to the mm2 psum
    during the drain copy as a per-partition scalar multiply, alternating
    DVE/ACT so neither engine chokes.
"""
from contextlib import ExitStack

import numpy as np

import concourse.bass as bass
import concourse.tile as tile
from concourse import bacc, mybir
from concourse.bass_utils import run_bass_kernel_spmd

F32 = mybir.dt.float32
BF16 = mybir.dt.bfloat16
AF = mybir.ActivationFunctionType

B, C, P, D, G = 8, 16, 512, 1024, 512
R = C * P            # 8192 rows per batch
MEGA = 512           # rows per mega-tile
NSUB = MEGA // 128   # 4
NMEGA = R // MEGA    # 16
NPAIR = NMEGA // 2   # 8 (qzT loads are 2 megas per tile: 2KB runs per (p,k))
KD = D // 128        # 8 d-chunks
KG = G // 128        # 4 g-chunks
EPS = 1e-6
N_CORES = 8


def build_kernel():
    nc = bacc.Bacc("TRN2", target_bir_lowering=False)
    qzT_d = nc.dram_tensor("qzT", [D, R], BF16, kind="ExternalInput")
    bw_d = nc.dram_tensor("bw", [G, D], BF16, kind="ExternalInput")
    bwT_d = nc.dram_tensor("bwT", [D, G], BF16, kind="ExternalInput")
    msg_d = nc.dram_tensor("msg", [R, D], BF16, kind="ExternalOutput")

    with tile.TileContext(nc) as tc, ExitStack() as ctx:
        const_pool = ctx.enter_context(tc.tile_pool(name="const", bufs=1))
        in_pool = ctx.enter_context(tc.tile_pool(name="inp", bufs=3))
        qgr_pool = ctx.enter_context(tc.tile_pool(name="qgrp", bufs=2))
        out_pool = ctx.enter_context(tc.tile_pool(name="outp", bufs=2))
        small_pool = ctx.enter_context(tc.tile_pool(name="smallp", bufs=2))
        qg_psum = ctx.enter_context(tc.tile_pool(name="qgps", bufs=2, space="PSUM"))
        msg_psum = ctx.enter_context(tc.tile_pool(name="msgps", bufs=6, space="PSUM"))

        ones_f = const_pool.tile([128, 1], F32)
        nc.vector.memset(ones_f, 1.0)
        ones_g = const_pool.tile([128, 1], BF16)
        nc.vector.tensor_copy(ones_g, ones_f)

        # Weights on the second HWDGE ring (nc.scalar). bwT in k-quarters:
        # finer slicing costs ~0.7us of ring issue time per extra DMA, which
        # outweighs the granularity win.
        bwT_sb = const_pool.tile([128, KD, G], BF16)
        bwT_view = bwT_d[:].rearrange("(k p) g -> p k g", p=128)
        for q in range(4):
            nc.scalar.dma_start(
                out=bwT_sb[:, 2 * q:2 * q + 2, :], in_=bwT_view[:, 2 * q:2 * q + 2, :]
            )
        # bw rides the sync ring BETWEEN pair0 and pair1 (see below): the two
        # HWDGE rings share the 16 SDMA engines, so anything queued early
        # halves the bandwidth of the pair0 load that gates the first mm1.
        bw_sb = const_pool.tile([128, KG, D], BF16)

        # ~10 junk matmuls on memset tiles bridge the DMA-bound load window
        # so the PE_HAM clock gate sees >=3.4us of sustained activity and the
        # real matmul stream starts at 2.4 GHz instead of warming up mid-way.
        warm_a = const_pool.tile([128, 128], BF16)
        nc.vector.memset(warm_a, 0.0)
        warm_b = const_pool.tile([128, 512], BF16)
        nc.vector.memset(warm_b, 0.0)
        # 16 MMs ~= 6.8us cold: enough to cover until the first qzT quarters
        # can physically arrive (~15us: 7.2us engine-start barrier + bwT+pair0
        # over the shared SDMA engines), without delaying the first real MM.
        warm_ps = msg_psum.tile([128, 512], F32, name="warm_ps", tag="m_ps")
        for _ in range(16):
            nc.tensor.matmul(warm_ps, warm_a, warm_b)

        def load_qzT(j):
            # one tile = 2 megas (1024 rows): per-(p,k) DRAM runs are 2KB.
            # pair 0 loads in single k-slices so the first matmul starts at
            # the earliest possible moment; later pairs are one DMA each.
            qzT = in_pool.tile([128, KD, 2 * MEGA], BF16, name="qzT")
            qzT_view = qzT_d[:, j * 2 * MEGA:(j + 1) * 2 * MEGA].rearrange(
                "(k p) r -> p k r", p=128
            )
            nq = 4 if j == 0 else 1
            step = KD // nq
            for q in range(nq):
                nc.sync.dma_start(
                    out=qzT[:, step * q:step * (q + 1), :],
                    in_=qzT_view[:, step * q:step * (q + 1), :],
                )
            return qzT

        pairs = {}

        def ensure_load(j):
            if 0 <= j < NPAIR and j not in pairs:
                pairs[j] = load_qzT(j)

        def mm1(t):
            # qgT[gc] = sum_k bwT[:,k,gc].T @ qzT[:,k,cols(t)] -> relu (ACT)
            qzT = pairs[t // 2]
            c0 = (t % 2) * MEGA
            qgr = qgr_pool.tile([128, KG, MEGA], BF16, name="qgr")
            for gc in range(KG):
                qg_ps = qg_psum.tile([128, MEGA], F32, name="qg_ps")
                for k in range(KD):
                    nc.tensor.matmul(
                        qg_ps,
                        bwT_sb[:, k, gc * 128:(gc + 1) * 128],
                        qzT[:, k, c0:c0 + MEGA],
                        start=(k == 0),
                        stop=(k == KD - 1),
                    )
                nc.scalar.activation(qgr[:, gc, :], qg_ps, AF.Relu)
            return qgr

        def mm2_block(t, qgr):
            msg_sb = out_pool.tile([128, NSUB, D], BF16, name="msg_sb")

            def mmgroup(s, h):
                m_ps = msg_psum.tile([128, 512], F32, name="m_ps")
                for gc in range(KG):
                    nc.tensor.matmul(
                        m_ps,
                        qgr[:, gc, s * 128:(s + 1) * 128],
                        bw_sb[:, gc, h * 512:(h + 1) * 512],
                        start=(gc == 0),
                        stop=(gc == KG - 1),
                    )
                return m_ps

            def drain(s, h, m_ps, sc_sb):
                # all drains on DVE: ACT only runs the relus, so a drain is
                # never queued behind the next mega's relus on ACT's strict
                # FIFO (that ordering stalled mm2 psum-slot reuse by ~3us)
                dst = msg_sb[:, s, h * 512:(h + 1) * 512]
                nc.vector.tensor_scalar_mul(dst, m_ps, sc_sb[:, s:s + 1])

            # rowsum over g, den-direct: DVE sums the 4 qgr chunks into
            # acc [128(g_low), p] (bf16, error ~1e-3 of den — negligible),
            # then 4 tiny N=1 matmuls acc_chunk.T @ ones produce den for
            # each 128-row sub ALREADY in per-partition column layout.
            # This replaces 4 N=512 rowsum MMs + 4 PE transposes + a DVE
            # copy (~1.2us of PE per mega) with ~0.35us of PE.
            ADD = mybir.AluOpType.add
            s1 = small_pool.tile([128, MEGA], BF16, name="acc_s1")
            nc.vector.scalar_tensor_tensor(s1, qgr[:, 0, :], 0.0, qgr[:, 1, :], ADD, ADD)
            s2 = small_pool.tile([128, MEGA], BF16, name="acc_s2")
            nc.vector.scalar_tensor_tensor(s2, qgr[:, 2, :], 0.0, qgr[:, 3, :], ADD, ADD)
            acc = small_pool.tile([128, MEGA], BF16, name="acc")
            nc.vector.scalar_tensor_tensor(acc, s1, 0.0, s2, ADD, ADD)

            pending = [(0, 0, mmgroup(0, 0))]

            # sc_ps lives in the msg pool: its slot's previous occupant was
            # drained a full mega ago. (In the qg pool it reused a slot whose
            # last reader is mm1(t+1)'s relu — a ~0.4us/mega PE stall.)
            sc_ps = msg_psum.tile([128, 512], F32, name="sc_ps", tag="m_ps")
            for ss in range(NSUB):
                nc.tensor.matmul(
                    sc_ps[:, ss:ss + 1],
                    acc[:, ss * 128:(ss + 1) * 128],
                    ones_g,
                )

            pending.append((0, 1, mmgroup(0, 1)))

            sc_sb = small_pool.tile([128, NSUB], F32, name="sc_sb")
            nc.vector.tensor_scalar_max(sc_sb, sc_ps[:, 0:NSUB], EPS)
            nc.vector.reciprocal(sc_sb, sc_sb)

            pending.append((1, 0, mmgroup(1, 0)))

            pending.append((1, 1, mmgroup(1, 1)))
            for (ps_, hs_, mp_) in pending:
                drain(ps_, hs_, mp_, sc_sb)

            last = t == NMEGA - 1
            if last:
                for s in (0, 1):
                    nc.sync.dma_start(
                        out=msg_d[t * MEGA + s * 128:t * MEGA + (s + 1) * 128, :],
                        in_=msg_sb[:, s, :],
                    )
            for s in (2, 3):
                for h in (0, 1):
                    drain(s, h, mmgroup(s, h), sc_sb)
                if last:
                    # per-sub stores at the end: the final store is only
                    # 256KB, shrinking the post-compute tail
                    nc.sync.dma_start(
                        out=msg_d[t * MEGA + s * 128:t * MEGA + (s + 1) * 128, :],
                        in_=msg_sb[:, s, :],
                    )
            if not last:
                # one store per mega: fewer ring-issue slots and completion
                # semaphores (the teardown epilogue scales with DMA count)
                nc.sync.dma_start(
                    out=msg_d[t * MEGA:(t + 1) * MEGA, :].rearrange(
                        "(s p) d -> p s d", p=128
                    ),
                    in_=msg_sb,
                )

        ensure_load(0)
        # bw is only needed by mm2(0) (~25us in); placing it after pair0 on
        # the sync ring keeps the first-mm1-gating load at full bandwidth.
        nc.sync.dma_start(
            out=bw_sb, in_=bw_d[:].rearrange("(gc p) d -> p gc d", p=128)
        )
        ensure_load(1)
        ensure_load(2)
        qgr_cur = mm1(0)
        for t in range(NMEGA):
            if t + 1 < NMEGA:
                if (t + 1) % 2 == 0:
                    ensure_load((t + 1) // 2 + 2)
                qgr_next = mm1(t + 1)
            else:
                qgr_next = None
            mm2_block(t, qgr_cur)
            qgr_cur = qgr_next

    nc.compile()
    return nc


_NC_CACHE = None


def _get_nc():
    global _NC_CACHE
    if _NC_CACHE is None:
        _NC_CACHE = build_kernel()
    return _NC_CACHE


def kernel(qz: np.ndarray, binary_weight: np.ndarray) -> np.ndarray:
    import ml_dtypes

    bf16 = ml_dtypes.bfloat16
    qz = np.asarray(qz, dtype=np.float32)
    bw32 = np.asarray(binary_weight, dtype=np.float32)
    assert qz.shape == (B, C, P, D), qz.shape
    assert bw32.shape == (B, G, D), bw32.shape
    bw = bw32.astype(bf16)

    nc = _get_nc()
    in_maps = []
    for i in range(N_CORES):
        qzT = np.ascontiguousarray(qz[i].reshape(R, D).T.astype(bf16))  # [D, R]
        bwT = np.ascontiguousarray(bw[i].T)                             # [D, G]
        in_maps.append({"qzT": qzT, "bw": bw[i], "bwT": bwT})
    res = run_bass_kernel_spmd(nc, in_maps, core_ids=list(range(N_CORES)))
    out = np.stack(
        [
            res.results[i]["msg"].astype(np.float32).reshape(C, P, D)
            for i in range(N_CORES)
        ],
        axis=0,
    )
    return out
